# revision 1
# baseline (speedup 1.0000x reference)
"""Trainium2 Bass kernel for nn_CrossAttention_29549374997155.

Computation (B=256, U=128, P=64, H=768):
  c[b,u,p] = cosine_sim(u_vec[b,u,:], p_vec[b,p,:])
  row_att = softmax(einsum('bup,oup->bo', c, w_utt) + b_utt)
  col_att = softmax(einsum('bup,opu->bo', c, w_pheno) + b_pheno)

Strategy: pure data parallel over batch (32 batches / core on 8 cores).
Host side: normalize rows (0.1% of FLOPs), lay out transposed operands so
the H contraction sits on SBUF partitions, pre-permute conv weights to
[u, p, o] with row/col output channels concatenated (o = 192), cast to
bf16. Device side per batch: 6 accumulating PE matmuls produce
c = unT.T @ pnT in PSUM; DVE copies it (fp32->bf16) into a persistent
C_all[u, p, batch] tile. Logits for all 32 batches then take 64
accumulating matmuls (contraction chunk = column p of c, stationary
C_all[:, p, :], moving weights [128, 192]) plus one K=1 matmul that adds
the bias via a ones row; the [32, 192] PSUM result has batches on
partitions so both softmaxes run along the free dim.
"""

import sys

if "/opt/trn_rl_repo" not in sys.path:
    sys.path.insert(0, "/opt/trn_rl_repo")

import ml_dtypes
import numpy as np

import concourse.bass as bass  # noqa: F401  (bass registers engine types)
import concourse.tile as tile
from concourse import bacc, mybir
from concourse.bass_utils import run_bass_kernel_spmd

B, U, P, H = 256, 128, 64, 768
NCORES = 8
NB = B // NCORES          # 32 batches per core
HC = H // 128             # 6 contraction chunks
O = U + P                 # 192 fused output channels
GRP = 4                   # batches per input DMA
EPS = 1e-8

IN_DT = mybir.dt.bfloat16
IN_NP = ml_dtypes.bfloat16

_CACHE = {}


def _build():
    nc = bacc.Bacc("TRN2", target_bir_lowering=False, debug=False)

    ut = nc.dram_tensor("ut", [128, NB, HC, U], IN_DT, kind="ExternalInput")
    pt = nc.dram_tensor("pt", [128, NB, HC, P], IN_DT, kind="ExternalInput")
    wt = nc.dram_tensor("wt", [U, P, O], IN_DT, kind="ExternalInput")
    bias = nc.dram_tensor("bias", [1, O], IN_DT, kind="ExternalInput")
    out = nc.dram_tensor("out", [NB, O], mybir.dt.float32, kind="ExternalOutput")

    f32 = mybir.dt.float32

    with tile.TileContext(nc) as tc:
        with (
            tc.tile_pool(name="u_in", bufs=3) as u_pool,
            tc.tile_pool(name="p_in", bufs=3) as p_pool,
            tc.tile_pool(name="singles", bufs=1) as singles,
            tc.tile_pool(name="cps", bufs=4, space="PSUM") as cps_pool,
            tc.tile_pool(name="lps", bufs=1, space="PSUM") as lps_pool,
            tc.tile_pool(name="sm", bufs=1) as sm_pool,
        ):
            wt_t = singles.tile([U, P, O], IN_DT)
            nc.sync.dma_start(out=wt_t[:], in_=wt.ap())
            bias_t = singles.tile([1, O], IN_DT)
            nc.sync.dma_start(out=bias_t[:], in_=bias.ap())
            ones_t = singles.tile([1, NB], IN_DT)
            nc.vector.memset(ones_t[:], 1.0)

            c_all = singles.tile([U, P, NB], IN_DT)

            ut_ap = ut.ap()
            pt_ap = pt.ap()
            for g in range(NB // GRP):
                u_t = u_pool.tile([128, GRP, HC, U], IN_DT)
                nc.sync.dma_start(
                    out=u_t[:], in_=ut_ap[:, g * GRP : (g + 1) * GRP, :, :]
                )
                p_t = p_pool.tile([128, GRP, HC, P], IN_DT)
                nc.sync.dma_start(
                    out=p_t[:], in_=pt_ap[:, g * GRP : (g + 1) * GRP, :, :]
                )
                for jj in range(GRP):
                    j = g * GRP + jj
                    ps_c = cps_pool.tile([U, P], f32)
                    for c in range(HC):
                        nc.tensor.matmul(
                            ps_c[:],
                            lhsT=u_t[:, jj, c, :],
                            rhs=p_t[:, jj, c, :],
                            start=(c == 0),
                            stop=(c == HC - 1),
                        )
                    nc.vector.tensor_copy(out=c_all[:, :, j], in_=ps_c[:])

            ps_l = lps_pool.tile([NB, O], f32)
            for p in range(P):
                nc.tensor.matmul(
                    ps_l[:],
                    lhsT=c_all[:, p, :],
                    rhs=wt_t[:, p, :],
                    start=(p == 0),
                    stop=False,
                )
            nc.tensor.matmul(
                ps_l[:], lhsT=ones_t[:], rhs=bias_t[:], start=False, stop=True
            )

            # two softmaxes along the free dim: [:, :U] rows, [:, U:] cols
            e_t = sm_pool.tile([NB, O], f32)
            out_t = sm_pool.tile([NB, O], f32)
            for lo, hi in ((0, U), (U, O)):
                neg_m = sm_pool.tile([NB, 1], f32, tag=f"negm{lo}")
                nc.vector.reduce_max(
                    out=neg_m[:], in_=ps_l[:, lo:hi],
                    axis=mybir.AxisListType.X, negate=True,
                )
                s_e = sm_pool.tile([NB, 1], f32, tag=f"sume{lo}")
                nc.scalar.activation(
                    out=e_t[:, lo:hi], in_=ps_l[:, lo:hi],
                    func=mybir.ActivationFunctionType.Exp,
                    bias=neg_m[:], scale=1.0, accum_out=s_e[:],
                )
                r_e = sm_pool.tile([NB, 1], f32, tag=f"rece{lo}")
                nc.vector.reciprocal(out=r_e[:], in_=s_e[:])
                nc.vector.tensor_scalar_mul(
                    out=out_t[:, lo:hi], in0=e_t[:, lo:hi], scalar1=r_e[:]
                )
            nc.sync.dma_start(out=out.ap(), in_=out_t[:])

    nc.compile()
    return nc


def _prep(utt_output, pheno_output, w_utt, b_utt, w_pheno, b_pheno):
    """Normalize, transpose and shard inputs on the host."""
    u = np.ascontiguousarray(np.swapaxes(np.asarray(utt_output), 0, 1))  # [B, U, H]
    p = np.ascontiguousarray(np.swapaxes(np.asarray(pheno_output), 0, 1))  # [B, P, H]
    un = u / np.maximum(np.linalg.norm(u, axis=-1, keepdims=True), EPS)
    pn = p / np.maximum(np.linalg.norm(p, axis=-1, keepdims=True), EPS)

    # wt[u, p, :U] = w_utt[o, u, p]; wt[u, p, U:] = w_pheno[o, p, u]
    wr = np.transpose(np.asarray(w_utt), (1, 2, 0))     # [U, P, U]
    wc = np.transpose(np.asarray(w_pheno), (2, 1, 0))   # [U, P, P]
    wt = np.ascontiguousarray(
        np.concatenate([wr, wc], axis=2), dtype=np.float32
    ).astype(IN_NP)
    bias = np.concatenate([np.asarray(b_utt), np.asarray(b_pheno)])
    bias = bias.reshape(1, O).astype(IN_NP)

    in_maps = []
    for i in range(NCORES):
        j0 = i * NB
        # [NB, U, H] -> [NB, U, HC, 128] -> (h_lo, j, c, u)
        ut_i = (
            un[j0 : j0 + NB]
            .reshape(NB, U, HC, 128)
            .transpose(3, 0, 2, 1)
        )
        pt_i = (
            pn[j0 : j0 + NB]
            .reshape(NB, P, HC, 128)
            .transpose(3, 0, 2, 1)
        )
        in_maps.append(
            {
                "ut": np.ascontiguousarray(ut_i, dtype=np.float32).astype(IN_NP),
                "pt": np.ascontiguousarray(pt_i, dtype=np.float32).astype(IN_NP),
                "wt": wt,
                "bias": bias,
            }
        )
    return in_maps


def _run(inputs, trace=False, trace_cores=None):
    if "nc" not in _CACHE:
        _CACHE["nc"] = _build()
    nc = _CACHE["nc"]
    in_maps = _prep(**inputs)
    res = run_bass_kernel_spmd(
        nc, in_maps, core_ids=list(range(NCORES)), trace=trace,
        trace_cores=trace_cores,
    )
    outs = [res.results[i]["out"] for i in range(NCORES)]
    row = np.concatenate([o[:, :U] for o in outs], axis=0).astype(np.float32)
    col = np.concatenate([o[:, U:] for o in outs], axis=0).astype(np.float32)
    return (row, col), res


def kernel(**inputs):
    (row, col), _ = _run(inputs, trace=False)
    return row, col
